# revision 64
# baseline (speedup 1.0000x reference)
"""Trainium2 Bass kernel for nn_Attention_16630113370932.

ViT-style attention block:
  x [64, 768, 14, 14] -> 1x1-conv qkv (w_qkv [2304, 768]) -> 12-head attention
  over N=196 tokens (head_dim 64, qk scale 64**-0.25 on both q and k)
  -> 1x1-conv proj (w_proj [768, 768]) -> out [64, 768, 14, 14]

Strategy: pure data-parallel over batch across 8 NeuronCores (8 images per
core, no collectives). All matmuls run in bf16 (fp32 PSUM accumulation);
weights are transposed + bf16-cast on the host. Attention per head computes
scores transposed, ST = K^T Q [m, n], exp on ScalarE, then a single PE pass
AV = [V | 1]^T exp(ST) yields both the weighted values and the softmax
denominator (accumulated across heads into one [12, N] PSUM tile via
one-hot matmuls); normalization is a batched VectorE reciprocal, a selector
matmul that broadcasts each head's reciprocal row onto its 64 output
partitions, and a VectorE multiply. Images are processed in pairs (392-wide
matmul free dim for qkv/proj); the two images' attention chains interleave at
head-pair granularity, and qkv/proj accumulation half-groups of neighboring
pairs are emitted as filler between attention steps to keep the PE dense.
"""
import numpy as np
import ml_dtypes

import concourse.bass as bass
import concourse.tile as tile
from concourse import mybir
from concourse.bass_utils import run_bass_kernel_spmd
from concourse.vector_clock import ScopedClock


def _patched_drain_and_barrier(self, tick_clock, wait_clock):
    """TileContext exit-drain carries one sem wait per global-clock proc; the
    walrus build in this image rejects >2 sync waits on a CTRL instruction
    ("Too many sync wait commands"). Keep one wait on the drain and spread the
    rest over single-wait nops on the same (SP) engine — equivalent ordering,
    since all of them precede the all-engine barrier."""
    nc = self.nc
    drain_inst = nc.sync.drain()
    wait_clock.add_sem_waits(
        drain_inst.ins, ScopedClock({None: tick_clock.global_clock})
    )
    si = drain_inst.ins.sync_info
    ow = list(si.on_wait or [])
    if len(ow) > 1:
        si.on_wait = ow[:1]
        for w in ow[1:]:
            nop = nc.sync.nop()
            nop.ins.sync_info = mybir.SyncInfo(on_wait=[w], on_update=[])
    nc.all_engine_barrier()
    assert self.sems is not None
    popped = nc._tile_sem_poison_stack.pop()
    assert popped is self._sem_poison
    nc.clear_and_free_semaphores(list(self.sems.allocated().values()))
    nc.all_engine_barrier()


tile.TileContext._drain_and_barrier = _patched_drain_and_barrier

_MAX_WAITS = 1  # walrus in this image rejects multiple sync waits per TPB instruction


def _split_sync_waits(m: dict) -> dict:
    """Move overflow sem-waits (beyond _MAX_WAITS) onto fresh NoOp
    instructions inserted just before the over-limit instruction on the same
    engine — same-engine program order makes this ordering-equivalent."""
    fresh = [0]
    for f in m.get("functions", []):
        for blk in f.get("blocks", []):
            ins_list = blk.get("instructions", [])
            out = []
            for ins in ins_list:
                si = ins.get("sync_info")
                ow = (si or {}).get("on_wait") or []
                if len(ow) > _MAX_WAITS:
                    eng = ins.get("engine")
                    extra = ow[: len(ow) - _MAX_WAITS]
                    si["on_wait"] = ow[len(ow) - _MAX_WAITS:]
                    for k in range(0, len(extra), _MAX_WAITS):
                        fresh[0] += 1
                        out.append({
                            "debug": ins.get("debug", 0),
                            "engine": eng,
                            "ins": [],
                            "name": f"I-waitsplit-{fresh[0]}",
                            "opcode": "NoOp",
                            "outs": [],
                            "sync_info": {
                                "on_update": [],
                                "on_wait": extra[k:k + _MAX_WAITS],
                            },
                        })
                out.append(ins)
            blk["instructions"] = out
    return m


_orig_to_json_bytes = bass.Bass.to_json_bytes


def _patched_to_json_bytes(self) -> bytes:
    import orjson

    m = orjson.loads(_orig_to_json_bytes(self))
    return orjson.dumps(_split_sync_waits(m))


bass.Bass.to_json_bytes = _patched_to_json_bytes

BF16 = mybir.dt.bfloat16
F32 = mybir.dt.float32

N_CORES = 8
B, CIN, HH, WW = 64, 768, 14, 14
HW = HH * WW            # 196 tokens
NH, DH = 12, 64         # heads, head dim
BPC = B // N_CORES      # 8 images per core
CT = CIN // 128         # 6 contraction tiles
SCALE2 = DH ** -0.5     # 0.125 == (dh**-0.25)**2, folded into w_q on host

_AF = mybir.ActivationFunctionType

# CoreSim rejects the fused exp's read of never-written PSUM rows (68:128 of
# the second score chunk — computed into SBUF but never consumed). Set True
# when building for the simulator to emit per-chunk exps instead.
SIM_SAFE = False


def build_kernel() -> bass.Bass:
    W2 = 2 * HW          # 392: free width of an image pair
    M1 = HW - 128        # 68: second token-chunk size
    nc = bass.Bass()
    xp = nc.declare_dram_parameter("xp", [CT, 128, BPC * HW], BF16, isOutput=False)
    wqkT = nc.declare_dram_parameter("wqkT", [CT, 128, 2304], BF16, isOutput=False)
    wpT = nc.declare_dram_parameter("wpT", [CT, 128, 768], BF16, isOutput=False)
    outp = nc.declare_dram_parameter("out", [CT, 128, BPC * HW], F32, isOutput=True)

    selp = nc.declare_dram_parameter("sel", [NH, CT * 128], BF16, isOutput=False)
    eyep = nc.declare_dram_parameter("eye", [128, NH * NH], BF16, isOutput=False)

    with tile.TileContext(nc) as tc:
        with (
            tc.tile_pool(name="weights", bufs=1) as wpool,
            tc.tile_pool(name="xin", bufs=1) as xpool,
            tc.tile_pool(name="qk", bufs=2) as qkpool,
            tc.tile_pool(name="vt", bufs=2) as vtpool,
            tc.tile_pool(name="exps", bufs=6) as epool,
            tc.tile_pool(name="rec", bufs=2) as recpool,
            tc.tile_pool(name="ao", bufs=2) as aopool,
            tc.tile_pool(name="osb", bufs=2) as opool,
            tc.tile_pool(name="psA", bufs=2, space="PSUM") as psA,
            tc.tile_pool(name="psST", bufs=2, space="PSUM") as psST,
            tc.tile_pool(name="psAV", bufs=2, space="PSUM") as psAV,
            tc.tile_pool(name="psDEN", bufs=2, space="PSUM") as psDEN,
        ):
            # tiny constants first (eyecols gates the first denominator
            # matmuls), then x and w_qk — the first qkv groups need them —
            # then w_p, which is only used once attention output exists.
            # eyecols[:, h, :] is the 12-col one-hot(h) matrix on every
            # partition: accumulates head h's softmax denominator into row h;
            # sel12[:, j, :].T @ rec12 broadcasts head 2j's recip row onto
            # partitions 0:64 and head 2j+1's onto 64:128.
            eyecols = wpool.tile([128, NH, NH], BF16)
            nc.sync.dma_start(eyecols.rearrange("p h c -> p (h c)"), eyep[:])
            sel12 = wpool.tile([NH, CT, 128], BF16)
            nc.sync.dma_start(sel12.rearrange("h j p -> h (j p)"), selp[:])
            x_sb = xpool.tile([128, CT, BPC * HW], BF16)
            w_qk = wpool.tile([128, CT, 2304], BF16)
            for t in range(CT):
                nc.sync.dma_start(x_sb[:, t, :], xp[t])
                nc.sync.dma_start(w_qk[:, t, :], wqkT[t])
            w_p = wpool.tile([128, CT, 768], BF16)
            for t in range(CT):
                nc.sync.dma_start(w_p[:, t, :], wpT[t])

            from collections import deque
            filler: deque = deque()

            def emit_filler(n):
                for _ in range(min(n, len(filler))):
                    filler.popleft()()

            def qkv_group(qk_sb, x0, j):
                state = {}

                def fn1():
                    state["ps"] = psA.tile([128, 512], F32, tag="psA",
                                           name=f"psq{x0}_{j}")
                    for t in range(CT // 2):
                        nc.tensor.matmul(
                            state["ps"][:, :W2],
                            lhsT=w_qk[:, t, j * 128:(j + 1) * 128],
                            rhs=x_sb[:, t, x0:x0 + W2],
                            start=(t == 0),
                            stop=False,
                        )

                def fn2():
                    ps = state["ps"]
                    for t in range(CT // 2, CT):
                        nc.tensor.matmul(
                            ps[:, :W2],
                            lhsT=w_qk[:, t, j * 128:(j + 1) * 128],
                            rhs=x_sb[:, t, x0:x0 + W2],
                            start=False,
                            stop=(t == CT - 1),
                        )
                    if j % 2 == 0:
                        nc.vector.tensor_copy(qk_sb[:, j, :], ps[:, :W2])
                    else:
                        nc.scalar.copy(qk_sb[:, j, :], ps[:, :W2])
                return fn1, fn2

            def vt_group(vt, xs, np0, nsz, of):
                def fn():
                    ps = psA.tile([128, 512], F32, tag="psA", name=f"psv{xs}_{np0}_{of}")
                    for t in range(CT):
                        nc.tensor.matmul(
                            ps[:nsz, :384],
                            lhsT=x_sb[:, t, xs + np0:xs + np0 + nsz],
                            rhs=w_qk[:, t, 1536 + of * 384:1536 + (of + 1) * 384],
                            start=(t == 0),
                            stop=(t == CT - 1),
                        )
                    nc.vector.tensor_copy(vt[:nsz, of * 6:(of + 1) * 6, :],
                                          ps[:nsz, :384])
                return fn

            def proj_group(out_sb, ao_n, x0, j, dma):
                state = {}

                def fn1():
                    state["ps"] = psA.tile([128, 512], F32, tag="psA",
                                           name=f"psp{x0}_{j}")
                    for t in range(CT // 2):
                        nc.tensor.matmul(
                            state["ps"][:, :W2],
                            lhsT=w_p[:, t, j * 128:(j + 1) * 128],
                            rhs=ao_n[:, t, :],
                            start=(t == 0),
                            stop=False,
                        )

                def fn2():
                    ps = state["ps"]
                    for t in range(CT // 2, CT):
                        nc.tensor.matmul(
                            ps[:, :W2],
                            lhsT=w_p[:, t, j * 128:(j + 1) * 128],
                            rhs=ao_n[:, t, :],
                            start=False,
                            stop=(t == CT - 1),
                        )
                    if j % 2 == 0:
                        nc.scalar.copy(out_sb[:, j, :], ps[:, :W2])
                    else:
                        nc.vector.tensor_copy(out_sb[:, j, :], ps[:, :W2])
                    if dma:
                        dst = outp.rearrange("t p n -> p t n")[:, :, x0:x0 + W2]
                        nc.sync.dma_start(dst, out_sb[:, :, :])
                return fn1, fn2

            qks = {}
            for pr in range(BPC // 2):
                x0 = pr * W2
                if pr == 0:
                    # first pair's q/k projection emitted directly
                    qks[0] = qkpool.tile([128, 2 * CT, W2], BF16, name="qk0",
                                         tag="qk")
                    for j in range(2 * CT):
                        for f in qkv_group(qks[0], 0, j):
                            f()

                qk_sb = qks[pr]
                ao_un = aopool.tile([128, CT, W2], BF16, tag="aoun")
                ao_n = aopool.tile([128, CT, W2], BF16, tag="aon")

                vts = {}
                for si in range(2):
                    for nch in range(2):
                        vts[(si, nch)] = vtpool.tile(
                            [128, NH, 64], BF16, tag=f"vt{si}_{nch}",
                            name=f"vt{si}{nch}p{pr}")
                den_l = [psDEN.tile([NH, HW], F32, tag="den",
                                    name=f"den{pr}_{u}") for u in range(2)]
                # vT for both images up front
                for si in range(2):
                    for nch, (np0, nsz) in enumerate(((0, 128), (128, M1))):
                        for of in range(2):
                            vt_group(vts[(si, nch)], x0 + si * HW, np0, nsz, of)()
                if pr + 1 < BPC // 2:
                    # next pair's q/k projection is the attention filler
                    qks[pr + 1] = qkpool.tile(
                        [128, 2 * CT, W2], BF16, name=f"qk{pr + 1}", tag="qk")
                    for j in range(2 * CT):
                        filler.extend(qkv_group(qks[pr + 1], (pr + 1) * W2, j))

                # ---- attention; the two images' chains interleave at
                # head-pair granularity (independent work hides each
                # chain's exp/evac latency). Heads pair into row groups
                # (ST) / col groups (AV); denominators accumulate into
                # per-image [12, HW] PSUM tiles via one-hot matmuls ----
                rec12 = recpool.tile([NH, W2], BF16, tag="rec12",
                                     name=f"rec{pr}")
                for hp in range(CT):
                    for si in range(2):
                        s0 = si * HW
                        vt0, vt1 = vts[(si, 0)], vts[(si, 1)]
                        h0, h1 = 2 * hp, 2 * hp + 1
                        # parity-interleaved emission: consecutive matmuls
                        # use disjoint PE row groups (partitions 0:64 vs
                        # 64:128), so they overlap and weight loads pull ahead
                        sts = [psST.tile([128, W2], F32, tag="st",
                                         name=f"st{pr}_{si}_{hp}_{u}")
                               for u in range(2)]
                        qs, ks = [], []
                        for h in (h0, h1):
                            po = (h % 2) * 64
                            qs.append(qk_sb[po:po + 64, hp, s0:s0 + HW])
                            ks.append(qk_sb[po:po + 64, CT + hp, s0:s0 + HW])
                        for u in range(2):
                            nc.tensor.matmul(sts[u][:, 0:HW],
                                             lhsT=ks[u][:, 0:128], rhs=qs[u],
                                             start=True, stop=True)
                        for u in range(2):
                            nc.tensor.matmul(sts[u][:M1, HW:W2],
                                             lhsT=ks[u][:, 128:HW], rhs=qs[u],
                                             start=True, stop=True)
                        exps_l = []
                        for st in sts:
                            exps = epool.tile([128, W2], BF16)
                            if SIM_SAFE:
                                nc.scalar.activation(exps[:, 0:HW],
                                                     st[:, 0:HW], _AF.Exp)
                                nc.scalar.activation(exps[:M1, HW:W2],
                                                     st[:M1, HW:W2], _AF.Exp)
                            else:
                                nc.scalar.activation(exps[:, :], st[:, :],
                                                     _AF.Exp)
                            exps_l.append(exps)
                        emit_filler(2)
                        av = psAV.tile([128, HW], F32, tag="av")
                        for h, exps in zip((h0, h1), exps_l):
                            po = (h % 2) * 64
                            nc.tensor.matmul(av[po:po + 64, :],
                                             lhsT=vt0[:, h, :],
                                             rhs=exps[:, 0:HW],
                                             start=True, stop=False)
                            nc.tensor.matmul(av[po:po + 64, :],
                                             lhsT=vt1[:M1, h, :],
                                             rhs=exps[:M1, HW:W2],
                                             start=False, stop=True)
                            nc.tensor.matmul(den_l[si][:],
                                             lhsT=eyecols[:, h, :],
                                             rhs=exps[:, 0:HW],
                                             start=(h == 0), stop=False)
                            nc.tensor.matmul(den_l[si][:],
                                             lhsT=eyecols[:M1, h, :],
                                             rhs=exps[:M1, HW:W2],
                                             start=False, stop=(h == NH - 1))
                        emit_filler(1)
                        nc.vector.tensor_copy(
                            ao_un[:, hp, s0:s0 + HW], av[:, :])

                # leftover queued work (next pair's q/k writes) must be
                # emitted before the next pair's attention reads it
                emit_filler(len(filler))
                with nc.allow_low_precision("softmax reciprocal in bf16"):
                    nc.vector.reciprocal(rec12[:, 0:HW], den_l[0][:])
                    nc.vector.reciprocal(rec12[:, HW:W2], den_l[1][:])

                def bc_mul(rec12, ao_n, ao_un, pr, j):
                    def fn():
                        bc_ps = psA.tile([128, 512], F32, tag="psA",
                                         name=f"psb{pr}_{j}")
                        nc.tensor.matmul(bc_ps[:, :W2], lhsT=sel12[:, j, :],
                                         rhs=rec12[:], start=True, stop=True)
                        nc.vector.tensor_mul(
                            ao_n[:, j, :], bc_ps[:, :W2], ao_un[:, j, :])
                    return fn

                for j in range(CT):
                    bc_mul(rec12, ao_n, ao_un, pr, j)()
                emit_filler(len(filler))

                # queue this pair's projection as filler for the next pair's
                # attention; last pair emits directly
                out_sb = opool.tile([128, CT, W2], F32, tag="osb",
                                    name=f"osb{pr}")
                if pr + 1 == BPC // 2:
                    emit_filler(len(filler))
                for j in range(CT):
                    fns = proj_group(out_sb, ao_n, x0, j, dma=(j == CT - 1))
                    if pr + 1 < BPC // 2:
                        filler.extend(fns)
                    else:
                        for f in fns:
                            f()
            emit_filler(len(filler))
    return nc


_NC_CACHE = None


def _get_nc():
    global _NC_CACHE
    if _NC_CACHE is None:
        _NC_CACHE = build_kernel()
    return _NC_CACHE


def prep_inputs(x: np.ndarray, w_qkv: np.ndarray, w_proj: np.ndarray):
    """Host-side shard + layout prep. Returns in_maps for the 8 cores."""
    bf16 = ml_dtypes.bfloat16
    w = w_qkv.astype(np.float64)
    w = np.concatenate([w[:768] * SCALE2, w[768:]], axis=0)  # fold qk scale into w_q
    wqkT = np.ascontiguousarray(w.T.astype(np.float32).astype(bf16)).reshape(CT, 128, 2304)
    wpT = np.ascontiguousarray(w_proj.T.astype(bf16)).reshape(CT, 128, 768)
    sel = np.zeros((NH, CT, 128), dtype=bf16)
    for j in range(CT):
        sel[2 * j, j, 0:64] = 1
        sel[2 * j + 1, j, 64:128] = 1
    sel = sel.reshape(NH, CT * 128)
    eye = np.broadcast_to(np.eye(NH, dtype=bf16), (128, NH, NH))
    eye = np.ascontiguousarray(eye).reshape(128, NH * NH)
    xr = x.reshape(B, CIN, HW)
    in_maps = []
    for c in range(N_CORES):
        xs = xr[c * BPC:(c + 1) * BPC]                      # [8, 768, 196]
        xs = xs.reshape(BPC, CT, 128, HW).transpose(1, 2, 0, 3)  # [6, 128, 8, 196]
        xs = np.ascontiguousarray(xs).astype(bf16).reshape(CT, 128, BPC * HW)
        in_maps.append({"xp": xs, "wqkT": wqkT, "wpT": wpT, "sel": sel, "eye": eye})
    return in_maps


def run(x, w_qkv, w_proj, trace=False, trace_kwargs=None):
    nc = _get_nc()
    in_maps = prep_inputs(x, w_qkv, w_proj)
    res = run_bass_kernel_spmd(
        nc, in_maps, core_ids=list(range(N_CORES)), trace=trace,
        **(trace_kwargs or {}),
    )
    outs = []
    for c in range(N_CORES):
        o = res.results[c]["out"].reshape(CT, 128, BPC, HW)   # f32
        o = o.transpose(2, 0, 1, 3).reshape(BPC, CIN, HH, WW)
        outs.append(o)
    full = np.concatenate(outs, axis=0).astype(np.float32)
    return full, res


def kernel(x: np.ndarray, w_qkv: np.ndarray, w_proj: np.ndarray) -> np.ndarray:
    out, _ = run(x, w_qkv, w_proj, trace=False)
    return out


if __name__ == "__main__":
    rng = np.random.default_rng(0)
    x = rng.standard_normal((B, CIN, HH, WW), dtype=np.float32)
    w_qkv = (rng.standard_normal((2304, 768), dtype=np.float32) * 0.02).astype(np.float32)
    w_proj = (rng.standard_normal((768, 768), dtype=np.float32) * 0.02).astype(np.float32)
    out = kernel(x, w_qkv, w_proj)
    print("out", out.shape, out.dtype)
